# revision 26
# baseline (speedup 1.0000x reference)
"""Self-contained Trainium2 Bass kernel for a dense transformer block.

Reference computation (per batch row):
  h  = LN(x; g1, b1);  q,k,v = per-head projections of h
  attn = softmax(causal(q k^T / sqrt(hs))) v;  x1 = x + concat(attn) Wp + bp
  h2 = LN(x1; g2, b2); out = x1 + gelu(h2 W1 + b1) W2 + b2

Shapes: x [4, 2048, 1024], 16 heads x 64, FFN 4096, fp32 in/out.

Sharding (8 cores, core = 2*b + g): core works on batch b, head group g
(8 heads) for attention, token half g for FFN.  All matmul operands are
bf16 (full PE rate, halved DMA/SBUF); statistics, PSUM, biases and
residuals stay fp32.

Schedule (single fused program):
  1. LN1 + transpose + V/K/Q projections, pipelined per 512-token block.
     LN stats via DVE reduce + Act square-accum; normalize fused into one
     Act op (per-partition scale+bias); no bn_stats.
  2. Attention with q-block-outer loop, order [0, 2, 1, 3].  Scores /
     mask / exp / AV restricted to the live width on diagonal blocks.
     QK(kt+1) issued before AV(kt) so the PE never waits on exp.
     attnT stays in SBUF.
  3. After q-blocks {0,2}: partial proj of token half A -> ReduceScatter A
     (bf16 payload) overlapped with remaining attention.  After {1,3}:
     proj half B -> RS B overlapped with FFN of half A.
  4. LN2 + FFN + residual per 512-token half; half A runs while RS B is
     still in flight.
"""

import numpy as np
from contextlib import ExitStack

B, T, C, H, HS, F = 4, 2048, 1024, 16, 64, 4096
EPS = 1e-5
N_CORES = 8
HG = H // 2          # heads per core
TH = T // 2          # tokens per core for LN2/FFN
CT = C // 128        # 8 c-tiles
FT = F // 128        # 32 f-tiles
QB = 512             # attention q-block width
NQB = T // QB        # 4
NT = T // 128        # 16 token tiles (full row)
NTH = TH // 128      # 8 token tiles (own half)
V65 = HG * 65        # v free width incl ones columns (520)

_CACHE = {}


def _build_nc(reps: int = 1, local: bool = False, phases: int = 6):
    import concourse.tile as tile
    from concourse import bacc, mybir

    f32 = mybir.dt.float32
    f32r = mybir.dt.float32r
    bf16 = mybir.dt.bfloat16
    i32 = mybir.dt.int32
    AF = mybir.ActivationFunctionType
    ALU = mybir.AluOpType

    nc = bacc.Bacc("TRN2", target_bir_lowering=False, debug=False,
                   num_devices=N_CORES)

    # ---- I/O (per-core views prepared on host) ----
    xb = nc.dram_tensor("xb", [T, C], bf16, kind="ExternalInput").ap()
    xres = nc.dram_tensor("xres", [TH, C], f32, kind="ExternalInput").ap()
    wq = nc.dram_tensor("wq", [C, HG * HS], bf16, kind="ExternalInput").ap()
    wk = nc.dram_tensor("wk", [C, HG * HS], bf16, kind="ExternalInput").ap()
    wv = nc.dram_tensor("wv", [C, V65], bf16, kind="ExternalInput").ap()
    bqk = nc.dram_tensor("bqk", [128, 8], f32, kind="ExternalInput").ap()
    bv = nc.dram_tensor("bv", [V65], f32, kind="ExternalInput").ap()
    wp = nc.dram_tensor("wp", [HG * HS, C], bf16, kind="ExternalInput").ap()
    w1 = nc.dram_tensor("w1", [FT, 128, CT, 128], bf16, kind="ExternalInput").ap()
    b1 = nc.dram_tensor("b1", [128, FT], f32, kind="ExternalInput").ap()
    w2 = nc.dram_tensor("w2", [F, C], bf16, kind="ExternalInput").ap()
    b2 = nc.dram_tensor("b2", [C], f32, kind="ExternalInput").ap()
    mask4 = nc.dram_tensor("mask4", [4, 128, QB], bf16, kind="ExternalInput").ap()
    ident = nc.dram_tensor("ident", [128, 128], bf16, kind="ExternalInput").ap()
    out = nc.dram_tensor("out", [TH, C], f32, kind="ExternalOutput").ap()

    xb_t = xb.rearrange("(tt p) c -> tt p c", p=128)
    xres_t = xres.rearrange("(tt p) c -> tt p c", p=128)
    out_t = out.rearrange("(tt p) c -> tt p c", p=128)
    w2_t = w2.rearrange("(ft p) c -> ft p c", p=128)
    wp_t = wp.rearrange("(ct p) c -> ct p c", p=128)
    wq_t = wq.rearrange("(ct p) m -> ct p m", p=128)
    wk_t = wk.rearrange("(ct p) m -> ct p m", p=128)
    wv_t = wv.rearrange("(ct p) m -> ct p m", p=128)

    import concourse.bass as bass

    def bcast_row(dram_ap, n_part, n_free):
        """DRAM [n_free] -> partition-broadcast AP [n_part, n_free]."""
        return bass.AP(tensor=dram_ap.tensor, offset=dram_ap.offset,
                       ap=[[0, n_part], [1, n_free]])

    with tile.TileContext(nc) as tc, ExitStack() as ctx:
        const = ctx.enter_context(tc.tile_pool(name="const", bufs=1))

        masks = const.tile([128, 4, QB], bf16)
        identity = const.tile([128, 128], bf16)
        nc.gpsimd.dma_start(identity, ident)
        bqk_s = const.tile([128, 8], f32)
        nc.gpsimd.dma_start(bqk_s, bqk)
        bv_s = const.tile([128, V65], f32)
        nc.gpsimd.dma_start(bv_s, bcast_row(bv, 128, V65))
        b1_s = const.tile([128, FT], f32)
        nc.gpsimd.dma_start(b1_s, b1)
        b2_s = const.tile([128, C], f32)
        nc.gpsimd.dma_start(b2_s, bcast_row(b2, 128, C))
        eps_s = const.tile([128, 1], f32)
        nc.vector.memset(eps_s, EPS)
        # f32 whose bits are the rsqrt magic 0x5f3759df
        magic_s = const.tile([128, 1], f32)
        nc.vector.memset(
            magic_s, float(np.uint32(0x5F3759DF).view(np.float32)))

        def emit_rsqrt(pool, v2):
            """rstd = 1/sqrt(v2/C + eps), DVE-only (no act-table traffic).

            Quake initial guess + 2 Newton steps; [128,1] ops are ~free.
            """
            u = pool.tile([128, 1], f32, tag="rs_u")
            nc.vector.tensor_scalar(u, v2, 1.0 / C, EPS, ALU.mult, ALU.add)
            yi = pool.tile([128, 1], i32, tag="rs_yi")
            nc.vector.tensor_scalar(yi, u.bitcast(i32), 1, None,
                                    ALU.logical_shift_right)
            nc.vector.tensor_tensor(out=yi, in0=magic_s.bitcast(i32),
                                    in1=yi, op=ALU.subtract)
            y = yi.bitcast(f32)
            t = pool.tile([128, 1], f32, tag="rs_t")
            for _ in range(2):
                nc.vector.tensor_tensor(out=t, in0=y, in1=y, op=ALU.mult)
                nc.vector.tensor_tensor(out=t, in0=t, in1=u, op=ALU.mult)
                nc.vector.tensor_scalar(t, t, -0.5, 1.5, ALU.mult, ALU.add)
                nc.vector.tensor_tensor(out=y, in0=y, in1=t, op=ALU.mult)
            return y

        for _rep in range(reps):
            pw_cm = tc.tile_pool(name="pw", bufs=1)
            pw = pw_cm.__enter__()
            vw_cm = tc.tile_pool(name="vw", bufs=1)
            vw = vw_cm.__enter__()
            ffn_cm = tc.tile_pool(name="ffn", bufs=1)
            ffn = ffn_cm.__enter__()
            lp2_cm = tc.tile_pool(name="ln2", bufs=2)
            lp2 = lp2_cm.__enter__()
            trps_cm = tc.tile_pool(name="trps", bufs=2, space="PSUM")
            trps = trps_cm.__enter__()
            kqp_cm = tc.tile_pool(name="kqp", bufs=1)
            kqp = kqp_cm.__enter__()
            v520 = kqp.tile([128, NT, V65], bf16)
            kT4 = kqp.tile([128, 4, T], bf16)
            qT4 = kqp.tile([128, 4, T], bf16)
            kw_cm = tc.tile_pool(name="kw", bufs=1)
            kw = kw_cm.__enter__()
            ph1_cm = tc.tile_pool(name="ph1", bufs=1)
            ph1 = ph1_cm.__enter__()
            xnT = ph1.tile([128, CT, T], bf16)

            # ---- Phase 1: LN1 -> xnT, then V + K/Q proj per 512-tok block ----
            with tc.tile_pool(name="ln1", bufs=3) as lp, \
                 tc.tile_pool(name="qkps", bufs=2, space="PSUM") as qkps:
                vps = qkps
                lps = trps
                # x tiles of the first block go first so LN1 starts
                # before the weight DMAs saturate HBM
                xt0s = []
                for i in range(4):
                    xt = lp.tile([128, C], bf16, tag="xt", bufs=4)
                    (nc.sync if i % 2 == 0 else nc.scalar).dma_start(
                        xt, xb_t[i])
                    xt0s.append(xt)
                wv_s = vw.tile([128, CT, V65], bf16)
                nc.gpsimd.dma_start(wv_s, wv_t.rearrange("ct p m -> p ct m"))
                wq_s = kw.tile([128, CT, HG * HS], bf16)
                nc.gpsimd.dma_start(wq_s, wq_t.rearrange("ct p m -> p ct m"))
                wk_s = kw.tile([128, CT, HG * HS], bf16)
                nc.gpsimd.dma_start(wk_s, wk_t.rearrange("ct p m -> p ct m"))
                for tb in range(NQB):
                    for i in range(4):
                        tt = tb * 4 + i
                        if tb == 0:
                            xt = xt0s[i]
                        else:
                            xt = lp.tile([128, C], bf16, tag="xt", bufs=4)
                            (nc.sync if tt % 2 == 0 else nc.scalar).dma_start(
                                xt, xb_t[tt])
                        s1 = lp.tile([128, 1], f32, tag="s1")
                        nc.vector.reduce_sum(s1, xt, axis=mybir.AxisListType.X)
                        sq = lp.tile([128, C], bf16, tag="sq", bufs=1)
                        s2 = lp.tile([128, 1], f32, tag="s2")
                        nc.scalar.activation(sq, xt, AF.Square, accum_out=s2)
                        v1 = lp.tile([128, 1], f32, tag="v1")
                        nc.vector.scalar_tensor_tensor(
                            out=v1, in0=s1, scalar=1.0 / C, in1=s1,
                            op0=ALU.mult, op1=ALU.mult)
                        v2 = lp.tile([128, 1], f32, tag="v2")
                        nc.vector.tensor_sub(v2, s2, v1)
                        rstd = emit_rsqrt(lp, v2)
                        nb = lp.tile([128, 1], f32, tag="nb")
                        nc.vector.scalar_tensor_tensor(
                            out=nb, in0=s1, scalar=-1.0 / C, in1=rstd,
                            op0=ALU.mult, op1=ALU.mult)
                        xn = lp.tile([128, C], bf16, tag="xn", bufs=2)
                        nc.scalar.activation(xn, xt, AF.Identity,
                                             bias=nb, scale=rstd)
                        for cg in range(2):
                            ps = lps.tile([128, 512], f32, tag="tr")
                            for j in range(4):
                                ct = cg * 4 + j
                                # plain matmul with identity moving = same
                                # transpose, but exempt from the
                                # transpose<->collective serialization
                                nc.tensor.matmul(
                                    ps[:, j * 128:(j + 1) * 128],
                                    xn[:, ct * 128:(ct + 1) * 128], identity,
                                    start=True, stop=True)
                            nc.vector.tensor_copy(
                                xnT[:, cg * 4:(cg + 1) * 4,
                                    tt * 128:(tt + 1) * 128],
                                ps.rearrange("p (c t) -> p c t", c=4))
                    # V projection for this block's 4 tiles
                    for i in range(4):
                        tt = tb * 4 + i
                        for half in range(2):
                            lo, hi = half * 260, (half + 1) * 260
                            ps = vps.tile([128, 260], f32, tag="vps")
                            for c2 in range(CT):
                                nc.tensor.matmul(
                                    ps, xnT[:, c2, tt * 128:(tt + 1) * 128],
                                    wv_s[:, c2, lo:hi],
                                    start=(c2 == 0), stop=(c2 == CT - 1))
                            nc.vector.tensor_tensor(
                                out=v520[:, tt, lo:hi], in0=ps,
                                in1=bv_s[:, lo:hi], op=ALU.add)
                    # K/Q projections for this block, all pairs
                    for pair in range(4):
                        for dst4, wsrc, bcol in ((qT4, wq_s, pair),
                                                 (kT4, wk_s, 4 + pair)):
                            ps = qkps.tile([128, QB], f32, tag="kq")
                            for c2 in range(CT):
                                nc.tensor.matmul(
                                    ps,
                                    wsrc[:, c2, pair * 128:(pair + 1) * 128],
                                    xnT[:, c2, tb * QB:(tb + 1) * QB],
                                    start=(c2 == 0), stop=(c2 == CT - 1))
                            nc.vector.tensor_scalar(
                                dst4[:, pair, tb * QB:(tb + 1) * QB], ps,
                                bqk_s[:, bcol:bcol + 1], None, ALU.add)

            ph1_cm.__exit__(None, None, None)
            kw_cm.__exit__(None, None, None)
            wp_s = pw.tile([128, 4, C], bf16)
            nc.gpsimd.dma_start(wp_s, wp_t.rearrange("ct p c -> p ct c"))
            if _rep == 0:
                nc.gpsimd.dma_start(masks, mask4.rearrange("j p q -> p j q"))

            # ---- Phase 2: attention (qb outer) + proj + ReduceScatter ----
            dram = ctx.enter_context(
                tc.tile_pool(name="dram", bufs=1, space="DRAM"))
            cc_in2 = [dram.tile([TH, C], bf16, name=f"cc_in{_rep}_{i}")
                      for i in range(2)]
            cc_out2 = [dram.tile([TH // 2, C], bf16, name=f"cc_out{_rep}_{i}")
                       for i in range(2)]

            atp_cm = tc.tile_pool(name="atp", bufs=1)
            atp = atp_cm.__enter__()
            attnT4 = atp.tile([128, 4, T], bf16)
            rbd = ctx.enter_context(
                tc.tile_pool(name="rbd", bufs=8, space="DRAM"))

            def emit_ln2(h5):
                """LN2 chain for one 512-token half -> x1h/x1nTh tiles."""
                x1h = ffn.tile([128, 4, C], f32, name=f"x1h_{_rep}_{h5}")
                x1nTh = ffn.tile([128, CT, 512], bf16,
                                 name=f"x1nT_{_rep}_{h5}")
                for i in range(4):
                    tl = h5 * 4 + i
                    pj = lp2.tile([128, C], bf16, tag="pj")
                    nc.scalar.dma_start(pj, cc_out2[h5][i * 128:(i + 1) * 128])
                    xr = lp2.tile([128, C], f32, tag="xr")
                    nc.scalar.dma_start(xr, xres_t[tl])
                    s1 = lp2.tile([128, 1], f32, tag="s1")
                    nc.vector.scalar_tensor_tensor(
                        out=x1h[:, i, :], in0=pj, scalar=1.0, in1=xr,
                        op0=ALU.mult, op1=ALU.add, accum_out=s1)
                    sq = lp2.tile([128, C], bf16, tag="sq", bufs=1)
                    s2 = lp2.tile([128, 1], f32, tag="s2")
                    nc.scalar.activation(sq, x1h[:, i, :], AF.Square,
                                         accum_out=s2)
                    v1 = lp2.tile([128, 1], f32, tag="v1")
                    nc.vector.scalar_tensor_tensor(
                        out=v1, in0=s1, scalar=1.0 / C, in1=s1,
                        op0=ALU.mult, op1=ALU.mult)
                    v2 = lp2.tile([128, 1], f32, tag="v2")
                    nc.vector.tensor_sub(v2, s2, v1)
                    rstd = emit_rsqrt(lp2, v2)
                    nb = lp2.tile([128, 1], f32, tag="nb")
                    nc.vector.scalar_tensor_tensor(
                        out=nb, in0=s1, scalar=-1.0 / C, in1=rstd,
                        op0=ALU.mult, op1=ALU.mult)
                    xn2 = lp2.tile([128, C], bf16, tag="xn2")
                    nc.scalar.activation(xn2, x1h[:, i, :], AF.Identity,
                                         bias=nb, scale=rstd)
                    for cg in range(2):
                        ps = trps.tile([128, 512], f32, tag="tr")
                        for j in range(4):
                            ct = cg * 4 + j
                            nc.tensor.matmul(
                                ps[:, j * 128:(j + 1) * 128],
                                xn2[:, ct * 128:(ct + 1) * 128], identity,
                                start=True, stop=True)
                        nc.vector.tensor_copy(
                            x1nTh[:, cg * 4:(cg + 1) * 4,
                                  i * 128:(i + 1) * 128],
                            ps.rearrange("p (c t) -> p c t", c=4))
                return x1h, x1nTh

            x1hs = [None, None]
            with tc.tile_pool(name="expp", bufs=4) as expp, \
                 tc.tile_pool(name="smal", bufs=6) as smal, \
                 tc.tile_pool(name="prj", bufs=3) as prj, \
                 tc.tile_pool(name="aps", bufs=2, space="PSUM") as aps, \
                 tc.tile_pool(name="avps", bufs=2, space="PSUM") as avps:
                for qb in (2, 0, 1, 3):
                    nkt = (qb + 1) * (QB // 128)
                    for pair in range(4):
                        pavs = [avps.tile([128, QB], f32, tag="avps",
                                          name=f"pav_{_rep}_{pair}_{qb}_{h}")
                                for h in range(2)]

                        def issue_qk(kt):
                            j = kt - qb * 4
                            w0 = j * 128 if j >= 0 else 0
                            pscore = aps.tile([128, 2, QB], f32, tag="scps")
                            for h in range(2):
                                nc.tensor.matmul(
                                    pscore[:, h, w0:QB],
                                    kT4[h * 64:h * 64 + 64, pair,
                                        kt * 128:(kt + 1) * 128],
                                    qT4[h * 64:h * 64 + 64, pair,
                                        qb * QB + w0:(qb + 1) * QB],
                                    start=True, stop=(j < 0))
                            if j >= 0:
                                for h in range(2):
                                    nc.tensor.matmul(
                                        pscore[:, h, w0:QB], identity,
                                        masks[:, j, w0:QB],
                                        start=False, stop=True)
                            et = expp.tile([128, 2, QB], bf16, tag="exp")
                            nc.scalar.activation(
                                et[:, :, w0:QB], pscore[:, :, w0:QB],
                                AF.Exp, bias=0.0, scale=HS ** -0.5)
                            return et, w0

                        def issue_av(kt, et, w0):
                            for h in range(2):
                                col = (pair * 2 + h) * 65
                                nc.tensor.matmul(
                                    pavs[h][0:65, w0:QB],
                                    v520[:, kt, col:col + 65],
                                    et[:, h, w0:QB],
                                    start=(kt == 0), stop=(kt == nkt - 1))

                        prev = None
                        for kt in range(nkt):
                            cur = issue_qk(kt)
                            if prev is not None:
                                issue_av(kt - 1, *prev)
                            prev = cur
                        issue_av(nkt - 1, *prev)

                        for h in range(2):
                            hp = h * 64
                            # evacuate PSUM promptly: the whole softmax
                            # normalize then runs from SBUF off the
                            # accumulation critical path
                            pav_s = smal.tile([65, QB], f32, tag="pav_s")
                            nc.vector.tensor_copy(pav_s, pavs[h][0:65, :])
                            recip = smal.tile([1, QB], f32, tag="recip")
                            nc.vector.reciprocal(recip, pav_s[64:65, :])
                            rb = rbd.tile([1, QB], f32, tag="rb")
                            nc.sync.dma_start(rb, recip)
                            rb_bc = smal.tile([64, QB], f32, tag="rb_bc")
                            nc.sync.dma_start(
                                rb_bc, bass.AP(tensor=rb.tensor,
                                               offset=rb.offset,
                                               ap=[[0, 64], [1, QB]]))
                            nc.vector.tensor_tensor(
                                out=attnT4[hp:hp + 64, pair,
                                           qb * QB:(qb + 1) * QB],
                                in0=pav_s[0:64, :], in1=rb_bc, op=ALU.mult)

                    # partial proj for this q-block -> cc_in
                    half = qb % 2
                    for i in range(4):
                        tt = qb * 4 + i
                        po = prj.tile([128, C], bf16, tag="po")
                        for nh in range(2):
                            ps = avps.tile([128, 512], f32, tag="avps",
                                           name=f"prps_{_rep}_{qb}_{i}_{nh}")
                            for pr in range(4):
                                nc.tensor.matmul(
                                    ps, attnT4[:, pr, tt * 128:(tt + 1) * 128],
                                    wp_s[:, pr, nh * 512:(nh + 1) * 512],
                                    start=(pr == 0), stop=(pr == 3))
                            nc.vector.tensor_copy(
                                po[:, nh * 512:(nh + 1) * 512], ps)
                        row = (qb // 2) * 512 + i * 128
                        nc.sync.dma_start(cc_in2[half][row:row + 128], po)
                    if qb in (0, 3):
                        h2 = qb % 2
                        if local:
                            nc.sync.dma_start(cc_out2[h2][:],
                                              cc_in2[h2][0:TH // 2])
                        else:
                            nc.gpsimd.collective_compute(
                                "ReduceScatter", ALU.add,
                                replica_groups=[[0, 1], [2, 3], [4, 5], [6, 7]],
                                ins=[cc_in2[h2][:]], outs=[cc_out2[h2][:]])
                    if qb == 3:
                        # RS-A landed long ago; proj-B covers the LN2-A
                        # pipeline fill, FFN-A follows immediately.  The
                        # wait_until hint stops the scheduler from hoisting
                        # the pj loads into the attention stream where they
                        # head-of-line-block the Act queue on RS-A.
                        with tc.tile_wait_until(_rep * 0.66 + 0.26):
                            x1hs[0] = emit_ln2(0)

            atp_cm.__exit__(None, None, None)
            kqp_cm.__exit__(None, None, None)

            if phases <= 3:
                nc.sync.dma_start(out[0:TH // 2],
                                  cc_out2[0][:].bitcast(f32)[:, 0:C // 2])
                trps_cm.__exit__(None, None, None)
                lp2_cm.__exit__(None, None, None)
                ffn_cm.__exit__(None, None, None)
                vw_cm.__exit__(None, None, None)
                pw_cm.__exit__(None, None, None)
                continue

            # ---- Phase 3: FFN + residual per half; LN2-B mid-FFN-A ----
            with tc.tile_pool(name="w1p", bufs=6) as w1p, \
                 tc.tile_pool(name="w2p", bufs=4) as w2p, \
                 tc.tile_pool(name="gst", bufs=2) as gst, \
                 tc.tile_pool(name="ost", bufs=6) as ost, \
                 tc.tile_pool(name="f1ps", bufs=2, space="PSUM") as f1ps, \
                 tc.tile_pool(name="f2ps", bufs=4, space="PSUM") as f2ps:
                for h5 in range(2):
                    x1h, x1nTh = x1hs[h5]
                    g = gst.tile([128, FT, 512], bf16, tag="g")
                    for ft in range(FT):
                        w1t = w1p.tile([128, CT, 128], bf16, tag="w1t")
                        nc.scalar.dma_start(w1t, w1[ft])
                        ps = f1ps.tile([128, 512], f32, tag="f1")
                        for c2 in range(CT):
                            nc.tensor.matmul(
                                ps, w1t[:, c2, :], x1nTh[:, c2, :],
                                start=(c2 == 0), stop=(c2 == CT - 1))
                        nc.scalar.activation(g[:, ft, :], ps, AF.Gelu,
                                             bias=b1_s[:, ft:ft + 1], scale=1.0)
                    if h5 == 0:
                        # half B's LN2: runs while FFN2-A owns the PE and
                        # RS-B lands; hint late so the pj-B load cannot
                        # block the FFN-A weight/gelu stream on RS-B
                        with tc.tile_wait_until(_rep * 0.66 + 0.40):
                            x1hs[1] = emit_ln2(1)
                    ots = [ost.tile([128, C], f32, tag="ot",
                                    name=f"ot_{_rep}_{h5}_{i}")
                           for i in range(4)]
                    for nh in range(2):
                        pss = [f2ps.tile([128, 512], f32, tag="f2",
                                         name=f"f2_{_rep}_{h5}_{nh}_{i}")
                               for i in range(4)]
                        for ft in range(FT):
                            w2t = w2p.tile([128, 512], bf16, tag="w2t")
                            nc.sync.dma_start(
                                w2t, w2_t[ft, :, nh * 512:(nh + 1) * 512])
                            for ts2 in range(4):
                                nc.tensor.matmul(
                                    pss[ts2],
                                    g[:, ft, ts2 * 128:(ts2 + 1) * 128],
                                    w2t, start=(ft == 0), stop=(ft == FT - 1))
                        for ts2 in range(4):
                            ot = ots[ts2]
                            nc.vector.scalar_tensor_tensor(
                                out=ot[:, nh * 512:(nh + 1) * 512],
                                in0=pss[ts2], scalar=1.0,
                                in1=x1h[:, ts2, nh * 512:(nh + 1) * 512],
                                op0=ALU.mult, op1=ALU.add)
                            nc.vector.tensor_tensor(
                                out=ot[:, nh * 512:(nh + 1) * 512],
                                in0=ot[:, nh * 512:(nh + 1) * 512],
                                in1=b2_s[:, nh * 512:(nh + 1) * 512],
                                op=ALU.add)
                            if nh == 1:
                                nc.scalar.dma_start(out_t[h5 * 4 + ts2], ot)

            trps_cm.__exit__(None, None, None)
            lp2_cm.__exit__(None, None, None)
            ffn_cm.__exit__(None, None, None)
            vw_cm.__exit__(None, None, None)
            pw_cm.__exit__(None, None, None)
    nc.compile()
    return nc


# ---------------------------------------------------------------------------
# Host-side input preparation
# ---------------------------------------------------------------------------

def _prepare_in_maps(inputs):
    import ml_dtypes
    b16 = ml_dtypes.bfloat16

    x = np.ascontiguousarray(np.asarray(inputs["x"], dtype=np.float32))
    Wq = np.asarray(inputs["Wq"], dtype=np.float32)
    Wk = np.asarray(inputs["Wk"], dtype=np.float32)
    Wv = np.asarray(inputs["Wv"], dtype=np.float32)
    Wp = np.asarray(inputs["Wp"], dtype=np.float32)
    bp = np.asarray(inputs["bp"], dtype=np.float32)
    W1 = np.asarray(inputs["W1"], dtype=np.float32)
    b1 = np.asarray(inputs["b1"], dtype=np.float32)
    W2 = np.asarray(inputs["W2"], dtype=np.float32)
    b2 = np.asarray(inputs["b2"], dtype=np.float32)
    g1 = np.asarray(inputs["g1"], dtype=np.float32)
    beta1 = np.asarray(inputs["beta1"], dtype=np.float32)
    g2 = np.asarray(inputs["g2"], dtype=np.float32)
    beta2 = np.asarray(inputs["beta2"], dtype=np.float32)

    # masks for the diagonal blocks, [j, k, q]: 0 where j*128 + k <= q
    kk = np.arange(128)[None, :, None]
    qq = np.arange(QB)[None, None, :]
    jj = np.arange(4)[:, None, None]
    mask4 = np.where(jj * 128 + kk <= qq, 0.0, -1e30).astype(b16)
    ident = np.eye(128, dtype=b16)

    # FFN weights with LN2 affine folded in
    W1s = g2[:, None] * W1                                  # [C, F]
    b1s = beta2 @ W1 + b1                                   # [F]
    w1_packed = np.ascontiguousarray(
        W1s.reshape(CT, 128, FT, 128).transpose(2, 1, 0, 3)).astype(b16)
    b1_packed = np.ascontiguousarray(b1s.reshape(FT, 128).T)  # [128, FT]

    per_g = []
    for g in range(2):
        hsel = slice(g * HG, (g + 1) * HG)
        # [C, HG*64], LN1 gamma folded
        wq_g = np.ascontiguousarray(
            (g1[:, None, None] * Wq[hsel].transpose(1, 0, 2))
            .reshape(C, -1)).astype(b16)
        wk_g = np.ascontiguousarray(
            (g1[:, None, None] * Wk[hsel].transpose(1, 0, 2))
            .reshape(C, -1)).astype(b16)
        # v with ones columns: [C, HG*65]
        wv_g = np.zeros((C, V65), dtype=np.float32)
        bv_g = np.zeros(V65, dtype=np.float32)
        for hh in range(HG):
            wv_g[:, hh * 65:hh * 65 + 64] = g1[:, None] * Wv[g * HG + hh]
            bv_g[hh * 65:hh * 65 + 64] = beta1 @ Wv[g * HG + hh]
            bv_g[hh * 65 + 64] = 1.0
        # beta1-fold biases for q/k, packed [128, 8]: cols 0-3 q, 4-7 k pairs
        bq_g = (beta1 @ Wq[hsel].reshape(-1, C).T).reshape(HG * HS)
        bk_g = (beta1 @ Wk[hsel].reshape(-1, C).T).reshape(HG * HS)
        bqk_g = np.concatenate(
            [bq_g.reshape(4, 128).T, bk_g.reshape(4, 128).T], axis=1)
        # proj rows for this head group
        wp_g = np.ascontiguousarray(
            Wp[g * HG * HS:(g + 1) * HG * HS]).astype(b16)
        per_g.append((wq_g, wk_g, wv_g.astype(b16), bv_g,
                      np.ascontiguousarray(bqk_g), wp_g))

    w2_b = W2.astype(b16)
    in_maps = []
    for core in range(N_CORES):
        b, g = divmod(core, 2)
        wq_g, wk_g, wv_g, bv_g, bqk_g, wp_g = per_g[g]
        xres = x[b, g * TH:(g + 1) * TH] + bp
        in_maps.append({
            "xb": x[b].astype(b16), "xres": np.ascontiguousarray(xres),
            "wq": wq_g, "wk": wk_g, "wv": wv_g, "bqk": bqk_g, "bv": bv_g,
            "wp": wp_g, "w1": w1_packed, "b1": b1_packed,
            "w2": w2_b, "b2": b2, "mask4": mask4, "ident": ident,
        })
    return in_maps


def _gather(results):
    out = np.empty((B, T, C), dtype=np.float32)
    for core in range(N_CORES):
        b, g = divmod(core, 2)
        out[b, g * TH:(g + 1) * TH] = results[core]["out"]
    return out


def kernel(**inputs) -> np.ndarray:
    from concourse.bass_utils import run_bass_kernel_spmd

    if "nc" not in _CACHE:
        _CACHE["nc"] = _build_nc()
    nc = _CACHE["nc"]
    in_maps = _prepare_in_maps(inputs)
    res = run_bass_kernel_spmd(nc, in_maps, core_ids=list(range(N_CORES)))
    return _gather(res.results)
